# revision 32
# baseline (speedup 1.0000x reference)
"""Trainium2 Bass kernel for CausalTemporalAttention.

Math (per batch b, position p; all ops pointwise in p except the k=3 conv):
  feat[w,c,p] = gelu(conv1d(u[w])[c,p] + emb_b[c])          (W=16, C=64, P=8192)
  last = feat[W-1]
  Q = q_w @ last + q_b
  scores[w,p] = sum_c K[w,c,p]*Q[c,p] / 8
              = (sum_c feat[w,c,p]*Qk[c,p] + sum_c k_b[c]*Q[c,p]) / 8
                where Qk = k_w^T @ Q            <- K projection eliminated
  attn = softmax_w(scores);   es = exp(s) = (1+tanh(s/2))/(1-tanh(s/2))
  ctxF[c,p] = sum_w attn[w,p]*feat[w,c,p]
  out = (o_w@v_w) @ ctxF + (o_w@v_b + o_b)     <- V projection eliminated
  x = out + last;  y = GroupNorm4(x)*gamma + beta

Sharding: data-parallel over B (8 batches -> 8 cores), no collectives.
"""

import numpy as np

import concourse.bass as bass
import concourse.tile as tile
from concourse import bacc, mybir
from concourse.bass_utils import run_bass_kernel_spmd
from concourse.masks import make_identity

F32 = mybir.dt.float32
BF16 = mybir.dt.bfloat16
AF = mybir.ActivationFunctionType
OP = mybir.AluOpType

B, W, P, C, G = 8, 16, 8192, 64, 4
EPS = 1e-5
PC = 2048          # positions per chunk
NS = 512           # matmul free-dim split (one PSUM bank fp32)
NCHUNK = P // PC
NPAIR = W // 2
NSPL = PC // NS

# pair -> (w_top, w_bot); pair 7 is (15, 14) so that `last` (w=15) sits in
# partitions 0:64 of its feat tile (matmul rhs must start at partition 0).
PAIR_W = [(2 * i, 2 * i + 1) for i in range(NPAIR - 1)] + [(15, 14)]


def build_core_kernel(ctx, tc, y_ap, ins, fat=BF16):
    """Emit the per-core program. ins: dict name -> DRAM AP. y_ap: (C, P) f32."""
    nc = tc.nc

    consts = ctx.enter_context(tc.tile_pool(name="consts", bufs=1))
    stage = ctx.enter_context(tc.tile_pool(name="stage", bufs=2))

    featp = ctx.enter_context(tc.tile_pool(name="featp", bufs=9))
    onep = ctx.enter_context(tc.tile_pool(name="onep", bufs=1))
    fatp = ctx.enter_context(tc.tile_pool(name="fatp", bufs=4))
    esbp = ctx.enter_context(tc.tile_pool(name="esbp", bufs=9))
    smallp = ctx.enter_context(tc.tile_pool(name="smallp", bufs=2))
    persist = ctx.enter_context(tc.tile_pool(name="persist", bufs=1))
    outp = ctx.enter_context(tc.tile_pool(name="outp", bufs=2))
    dramp = ctx.enter_context(tc.tile_pool(name="dramp", bufs=2, space="DRAM"))

    pp_conv = ctx.enter_context(tc.tile_pool(name="pp_conv", bufs=2, space="PSUM"))
    pp_mm = ctx.enter_context(tc.tile_pool(name="pp_mm", bufs=2, space="PSUM"))
    pp_sc = ctx.enter_context(tc.tile_pool(name="pp_sc", bufs=2, space="PSUM"))
    pp_tp = ctx.enter_context(tc.tile_pool(name="pp_tp", bufs=2, space="PSUM"))

    # ---------------- constants / weights prep ----------------
    # conv lhsT block (6, 128): row j*3+t, col w'*64+c = emb_w[c,t] if w'==j
    # else 0; replicated at partition strips 0/32/64/96 (rhs/lhsT partition
    # bases must be 32-aligned for compute engines).
    convW4 = consts.tile([128, 128], F32, tag="convW4")
    nc.vector.memset(convW4[:], 0.0)
    emb_tc = ins["emb_w"].rearrange("c one t -> (one t) c")  # (3, 64) strided
    for q in range(4):
        for j in range(2):
            nc.sync.dma_start(
                out=convW4[32 * q + 3 * j : 32 * q + 3 * j + 3, 64 * j : 64 * j + 64],
                in_=emb_tc,
            )

    # identity (used by PE transposes)
    I128 = consts.tile([128, 128], F32, tag="I128")
    make_identity(nc, I128[:])

    def load_small(name, shape, tag):
        t = consts.tile(shape, F32, tag=tag)
        src = ins[name]
        if len(shape) == 2 and shape[1] == 1 and len(src.shape) == 1:
            src = src.unsqueeze(1)
        nc.sync.dma_start(out=t[:], in_=src)
        return t

    qb1 = load_small("q_b", [C, 1], "qb1")
    kb1 = load_small("k_b", [C, 1], "kb1")
    vb1 = load_small("v_b", [C, 1], "vb1")
    ob1 = load_small("o_b", [C, 1], "ob1")
    gamma1 = load_small("gn_gamma", [C, 1], "gamma1")
    beta1 = load_small("gn_beta", [C, 1], "beta1")
    vw_sb = load_small("v_w", [C, C], "vw")  # v_w[m, c] as-is

    embB2 = consts.tile([128, 1], F32, tag="embB2")
    for j in range(2):
        nc.sync.dma_start(out=embB2[64 * j : 64 * j + 64, :], in_=ins["emb_b"].unsqueeze(1))

    # q_w^T / o_w^T via strided DMA (tiny, one-time)
    qwT_f = stage.tile([C, C], F32, tag="st64")
    nc.sync.dma_start(out=qwT_f[:], in_=ins["q_w"].rearrange("o c -> c o"))
    lhsT_q = consts.tile([C, C], fat, tag="lhsT_q")
    nc.vector.tensor_copy(out=lhsT_q[:], in_=qwT_f[:])

    owT_f = consts.tile([C, C], F32, tag="owT")
    nc.sync.dma_start(out=owT_f[:], in_=ins["o_w"].rearrange("o c -> c o"))

    # lhsT_k (64, 128) = k_w columns duplicated: lhsT_k[c, j*64+o] = k_w[c, o]
    kw_dup_f = stage.tile([C, 128], F32, tag="st128")
    nc.sync.dma_start(
        out=kw_dup_f[:], in_=ins["k_w"].unsqueeze(1).to_broadcast([C, 2, C])
    )
    lhsT_k = consts.tile([C, 128], fat, tag="lhsT_k")
    nc.vector.tensor_copy(out=lhsT_k[:], in_=kw_dup_f[:])

    # kb16 (64, 16): k_b broadcast along free (memset + per-partition scalar add)
    kb16 = consts.tile([C, W], fat, tag="kb16")
    nc.vector.memset(kb16[:], 0.0)
    nc.vector.tensor_scalar(
        out=kb16[:], in0=kb16[:], scalar1=kb1[:], scalar2=None, op0=OP.add
    )

    # per-pair scores selector (128, 16)
    ones16 = []
    for i in range(NPAIR):
        wt, wb = PAIR_W[i]
        t = consts.tile([128, W], fat, tag=f"ones16_{i}")
        nc.vector.memset(t[:], 0.0)
        nc.vector.memset(t[0:64, wt : wt + 1], 1.0)
        nc.vector.memset(t[64:128, wb : wb + 1], 1.0)
        ones16.append(t)

    # pair-sum reduction matrix [I64; I64] (128, 64)
    sumI = consts.tile([128, C], fat, tag="sumI")
    nc.vector.tensor_copy(out=sumI[0:64, :], in_=I128[0:64, 0:64])
    nc.vector.tensor_copy(out=sumI[64:128, :], in_=I128[64:128, 64:128])

    # OV^T = (o_w @ v_w)^T : OVT[c,o] = sum_m v_w[m,c] * o_w[o,m]
    ovT_ps = pp_mm.tile([C, C], F32, tag="mm")
    nc.tensor.matmul(out=ovT_ps[:], lhsT=vw_sb[:], rhs=owT_f[:], start=True, stop=True)
    ovT = consts.tile([C, C], fat, tag="ovT")
    nc.vector.tensor_copy(out=ovT[:], in_=ovT_ps[:])

    # x shift vector: o_w @ v_b + o_b
    ovb_ps = pp_mm.tile([C, 1], F32, tag="mm")
    nc.tensor.matmul(out=ovb_ps[:], lhsT=owT_f[:], rhs=vb1[:], start=True, stop=True)
    sxv = consts.tile([C, 1], F32, tag="sxv")
    nc.vector.tensor_add(out=sxv[:], in0=ovb_ps[:], in1=ob1[:])

    # group-combine matrices for GroupNorm
    CG = C // G
    # gsel[c, g] = 1/CG where c//CG == g (via two affine selects: 0<=c-CG*g<CG)
    gsel = consts.tile([C, G], F32, tag="gsel")
    nc.gpsimd.memset(gsel[:], 1.0 / CG)
    nc.gpsimd.affine_select(
        out=gsel[:], in_=gsel[:], compare_op=OP.is_ge, fill=0.0,
        base=0, channel_multiplier=1, pattern=[[-CG, G]],
    )
    nc.gpsimd.affine_select(
        out=gsel[:], in_=gsel[:], compare_op=OP.is_ge, fill=0.0,
        base=CG - 1, channel_multiplier=-1, pattern=[[CG, G]],
    )
    # gselT[g, c] = 1.0 where c//CG == g
    gselT = consts.tile([G, C], F32, tag="gselT")
    nc.gpsimd.memset(gselT[:], 1.0)
    nc.gpsimd.affine_select(
        out=gselT[:], in_=gselT[:], compare_op=OP.is_ge, fill=0.0,
        base=0, channel_multiplier=-CG, pattern=[[1, C]],
    )
    nc.gpsimd.affine_select(
        out=gselT[:], in_=gselT[:], compare_op=OP.is_ge, fill=0.0,
        base=CG - 1, channel_multiplier=CG, pattern=[[-1, C]],
    )

    # persistent accumulators
    x_all = persist.tile([C, P], F32, tag="x_all")
    stats_all = persist.tile([C, NCHUNK * NSPL, 6], F32, tag="stats_all")
    # u staging tile (persistent; strip rows 6..31 stay zero so the rounded
    # 32-row matmul strip reads defined zeros that the zero lhsT rows kill)
    ut = persist.tile([128, 2 * PC], F32, tag="ut")
    nc.vector.memset(ut[:], 0.0)

    # u padded with one zero column on each side (conv pad=1), staged in DRAM
    u_pad = dramp.tile([W, P + 2], F32, tag="u_pad")
    zcol = consts.tile([W, 1], F32, tag="zcol")
    nc.vector.memset(zcol[:], 0.0)
    nc.sync.dma_start(out=u_pad[:, 0:1], in_=zcol[:])
    nc.sync.dma_start(out=u_pad[:, P + 1 : P + 2], in_=zcol[:])
    nc.sync.dma_start(out=u_pad[:, 1 : P + 1], in_=ins["u_history"])
    u = u_pad[:]  # (W, P+2); u_pad[:, p+1] == u[:, p]

    # ---------------- main loop over P chunks ----------------
    for ck in range(NCHUNK):
        p0 = ck * PC

        # one u tile: pair i at partition strip 32*(i%4), cols (i//4)*PC,
        # rows 32q + 3j + t = u_pad[w(i,j), p0+p+t]  (pad shift: u[p] = u_pad[p+1])
        # One DMA per (pair, j): dest = 3 contiguous partitions (taps), src AP
        # partition dim = tap shift (step 1 in u_pad's free axis).
        for i in range(NPAIR):
            a, q = divmod(i, 4)
            for j in range(2):
                wij = PAIR_W[i][j]
                src = bass.AP(
                    tensor=u.tensor,
                    offset=u.offset + wij * (P + 2) + p0,
                    ap=[[1, 3], [1, PC]],
                )
                nc.sync.dma_start(
                    out=ut[32 * q + 3 * j : 32 * q + 3 * j + 3, a * PC : (a + 1) * PC],
                    in_=src,
                )

        # conv + gelu; emit pair 7 first so Q/Qk can start early
        feat = [None] * NPAIR
        pair_order = [7] + list(range(7))
        for i in pair_order:
            a, q = divmod(i, 4)
            ft = featp.tile([128, PC], fat, tag="feat")
            for k in range(NSPL):
                cps = pp_conv.tile([128, NS], F32, tag="conv")
                nc.tensor.matmul(
                    out=cps[:],
                    lhsT=convW4[32 * q : 32 * q + 6, :],
                    rhs=ut[32 * q : 32 * q + 6, a * PC + k * NS : a * PC + (k + 1) * NS],
                    start=True,
                    stop=True,
                    tile_position=(32 * q, 0),
                )
                nc.scalar.activation(
                    out=ft[:, k * NS : (k + 1) * NS],
                    in_=cps[:],
                    func=AF.Gelu,
                    bias=embB2[:],
                    scale=1.0,
                )
            feat[i] = ft

        # Q = q_w @ last + q_b  (A), then Qk2 = [k_w^T A ; k_w^T A]
        A_sb = smallp.tile([C, PC], fat, tag="A_sb")
        qk2 = fatp.tile([128, PC], fat, tag="qk2")
        for k in range(NSPL):
            aps = pp_mm.tile([C, NS], F32, tag="mm")
            nc.tensor.matmul(
                out=aps[:],
                lhsT=lhsT_q[:],
                rhs=feat[7][0:64, k * NS : (k + 1) * NS],
                start=True,
                stop=True,
            )
            nc.vector.tensor_scalar(
                out=A_sb[:, k * NS : (k + 1) * NS],
                in0=aps[:],
                scalar1=qb1[:],
                scalar2=None,
                op0=OP.add,
            )
        for k in range(NSPL):
            qps = pp_mm.tile([128, NS], F32, tag="mm")
            nc.tensor.matmul(
                out=qps[:],
                lhsT=lhsT_k[:],
                rhs=A_sb[:, k * NS : (k + 1) * NS],
                start=True,
                stop=True,
            )
            nc.any.tensor_copy(out=qk2[:, k * NS : (k + 1) * NS], in_=qps[:])

        # prod = feat[i] * qk2 (per split) ; scores via accumulating 16-col matmuls
        s_sb = onep.tile([W, PC], F32, tag="s_sb")
        for k in range(NSPL):
            sl = slice(k * NS, (k + 1) * NS)
            scps = pp_sc.tile([W, NS], F32, tag="sc")
            for i in range(NPAIR):
                pr = fatp.tile([128, NS], fat, tag="prod")
                nc.vector.tensor_mul(out=pr[:], in0=feat[i][:, sl], in1=qk2[:, sl])
                nc.tensor.matmul(
                    out=scps[:],
                    lhsT=ones16[i][:],
                    rhs=pr[:],
                    start=(i == 0),
                    stop=False,
                )
            nc.tensor.matmul(
                out=scps[:],
                lhsT=kb16[:],
                rhs=A_sb[:, sl],
                start=False,
                stop=True,
            )
            nc.any.tensor_copy(out=s_sb[:, sl], in_=scps[:])

        # transpose scores to (128, 16 slabs, 16 w) and do softmax there
        sT_ps = pp_tp.tile([128, W * W], F32, tag="tp")
        for s in range(W):
            nc.tensor.transpose(
                out=sT_ps[:, W * s : W * (s + 1)],
                in_=s_sb[:, 128 * s : 128 * (s + 1)],
                identity=I128[0:16, 0:16],
            )
        # es = exp(s/8) = (1+th)/(1-th), th = tanh(s/16)
        th = smallp.tile([128, W * W], F32, tag="th")
        nc.scalar.activation(out=th[:], in_=sT_ps[:], func=AF.Tanh, scale=1.0 / 16.0)
        num = smallp.tile([128, W * W], F32, tag="num")
        nc.scalar.activation(out=num[:], in_=th[:], func=AF.Identity, bias=1.0)
        den = smallp.tile([128, W * W], F32, tag="den")
        nc.scalar.activation(out=den[:], in_=th[:], func=AF.Identity, bias=1.0, scale=-1.0)
        nc.vector.reciprocal(out=den[:], in_=den[:])
        es = num  # es = (1+th)*rden, in place
        nc.vector.tensor_mul(out=es[:], in0=num[:], in1=den[:])
        es3 = es[:].rearrange("p (t w) -> p t w", w=W)
        D = smallp.tile([128, W], F32, tag="D")
        nc.vector.tensor_reduce(out=D[:], in_=es3, axis=mybir.AxisListType.X, op=OP.add)
        nc.vector.reciprocal(out=D[:], in_=D[:])
        attnT = es  # normalize in place
        nc.vector.tensor_mul(
            out=attnT[:].rearrange("p (t w) -> p t w", w=W),
            in0=es3,
            in1=D[:].unsqueeze(2).to_broadcast([128, W, W]),
        )

        # transpose attn back to (16, PC), stage to DRAM for broadcast
        attn_sb = smallp.tile([W, PC], fat, tag="attn_sb")
        for k in range(NSPL):
            atps = pp_tp.tile([W, NS], F32, tag="tp")
            for s in range(4):
                g = 4 * k + s
                nc.tensor.transpose(
                    out=atps[:, 128 * s : 128 * (s + 1)],
                    in_=attnT[:, W * g : W * (g + 1)],
                    identity=I128[:],
                )
            nc.any.tensor_copy(out=attn_sb[:, k * NS : (k + 1) * NS], in_=atps[:])
        attn_dram = dramp.tile([W, PC], fat, tag="attn_dram")
        nc.sync.dma_start(out=attn_dram[:], in_=attn_sb[:])

        # ctxF = sum_w attn_w * feat_w  (broadcast attn rows via DMA, multiply,
        # then pair-sum with accumulating [I64;I64] matmuls)
        ctxF = smallp.tile([C, PC], fat, tag="ctxF")
        esBs = []
        for i in range(NPAIR):
            wt, wb = PAIR_W[i]
            esB = esbp.tile([128, PC], fat, tag="esB")
            nc.sync.dma_start(
                out=esB[0:64, :],
                in_=attn_dram[wt : wt + 1, :].to_broadcast([64, PC]),
            )
            nc.sync.dma_start(
                out=esB[64:128, :],
                in_=attn_dram[wb : wb + 1, :].to_broadcast([64, PC]),
            )
            esBs.append(esB)
        for k in range(NSPL):
            sl = slice(k * NS, (k + 1) * NS)
            cxps = pp_mm.tile([C, NS], F32, tag="mm")
            for i in range(NPAIR):
                p2 = fatp.tile([128, NS], fat, tag="prod2")
                nc.vector.tensor_mul(out=p2[:], in0=feat[i][:, sl], in1=esBs[i][:, sl])
                nc.tensor.matmul(
                    out=cxps[:],
                    lhsT=sumI[:],
                    rhs=p2[:],
                    start=(i == 0),
                    stop=(i == NPAIR - 1),
                )
            nc.any.tensor_copy(out=ctxF[:, sl], in_=cxps[:])

        # out = OV @ ctxF ; x = out + sxv + last ; bn stats
        for k in range(NSPL):
            ops_ = pp_mm.tile([C, NS], F32, tag="mm")
            nc.tensor.matmul(
                out=ops_[:],
                lhsT=ovT[:],
                rhs=ctxF[:, k * NS : (k + 1) * NS],
                start=True,
                stop=True,
            )
            xsl = x_all[:, p0 + k * NS : p0 + (k + 1) * NS]
            nc.vector.scalar_tensor_tensor(
                out=xsl,
                in0=ops_[:],
                scalar=sxv[:],
                in1=feat[7][0:64, k * NS : (k + 1) * NS],
                op0=OP.add,
                op1=OP.add,
            )
            nc.vector.bn_stats(out=stats_all[:, ck * NSPL + k, :], in_=xsl)

    # ---------------- GroupNorm finale ----------------
    mv = smallp.tile([C, 2], F32, tag="mv")
    nc.vector.bn_aggr(out=mv[:], in_=stats_all[:])
    # me2 = [mean, E[x^2]] per channel; group stats = 1/CG average via gsel
    me2 = smallp.tile([C, 2], F32, tag="me2")
    nc.any.tensor_copy(out=me2[:, 0:1], in_=mv[:, 0:1])
    sq = smallp.tile([C, 1], F32, tag="sq")
    nc.vector.tensor_mul(out=sq[:], in0=mv[:, 0:1], in1=mv[:, 0:1])
    nc.vector.tensor_add(out=me2[:, 1:2], in0=mv[:, 1:2], in1=sq[:])
    gm_ps = pp_mm.tile([G, 2], F32, tag="mm")
    nc.tensor.matmul(out=gm_ps[:], lhsT=gsel[:], rhs=me2[:], start=True, stop=True)
    gm = smallp.tile([G, 2], F32, tag="gm")
    nc.any.tensor_copy(out=gm[:], in_=gm_ps[:])
    # rstd = 1/sqrt(var+eps); var = Ex2G - meanG^2  (use sqrt(eps - (m^2-Ex2)))
    nvar = smallp.tile([G, 1], F32, tag="nvar")
    nc.vector.scalar_tensor_tensor(
        out=nvar[:],
        in0=gm[:, 0:1],
        scalar=gm[:, 0:1],
        in1=gm[:, 1:2],
        op0=OP.mult,
        op1=OP.subtract,
    )
    eps_t = smallp.tile([G, 1], F32, tag="eps_t")
    nc.vector.memset(eps_t[:], EPS)
    std = smallp.tile([G, 1], F32, tag="std")
    nc.scalar.activation(out=std[:], in_=nvar[:], func=AF.Sqrt, scale=-1.0, bias=eps_t[:])
    rstd = smallp.tile([G, 1], F32, tag="rstd")
    nc.vector.reciprocal(out=rstd[:], in_=std[:])
    # broadcast group [mean, rstd] to channels
    grs = smallp.tile([G, 2], F32, tag="grs")
    nc.any.tensor_copy(out=grs[:, 0:1], in_=gm[:, 0:1])
    nc.any.tensor_copy(out=grs[:, 1:2], in_=rstd[:])
    chb_ps = pp_mm.tile([C, 2], F32, tag="mm")
    nc.tensor.matmul(out=chb_ps[:], lhsT=gselT[:], rhs=grs[:], start=True, stop=True)
    chb = smallp.tile([C, 2], F32, tag="chb")
    nc.any.tensor_copy(out=chb[:], in_=chb_ps[:])
    scale_v = smallp.tile([C, 1], F32, tag="scale_v")
    nc.vector.tensor_mul(out=scale_v[:], in0=chb[:, 1:2], in1=gamma1[:])
    nmean = smallp.tile([C, 1], F32, tag="nmean")
    nc.vector.tensor_mul(out=nmean[:], in0=chb[:, 0:1], in1=scale_v[:])
    bias_v = smallp.tile([C, 1], F32, tag="bias_v")
    nc.vector.tensor_sub(out=bias_v[:], in0=beta1[:], in1=nmean[:])

    for ck in range(NCHUNK):
        for k in range(NSPL):
            off = ck * PC + k * NS
            ysl = outp.tile([C, NS], F32, tag="y_sb")
            nc.vector.tensor_scalar(
                out=ysl[:],
                in0=x_all[:, off : off + NS],
                scalar1=scale_v[:],
                scalar2=bias_v[:],
                op0=OP.mult,
                op1=OP.add,
            )
            nc.sync.dma_start(out=y_ap[:, off : off + NS], in_=ysl[:])


INPUT_SPECS = {
    "u_history": (W, P),
    "emb_w": (C, 1, 3),
    "emb_b": (C,),
    "q_w": (C, C),
    "q_b": (C,),
    "k_w": (C, C),
    "k_b": (C,),
    "v_w": (C, C),
    "v_b": (C,),
    "o_w": (C, C),
    "o_b": (C,),
    "gn_gamma": (C,),
    "gn_beta": (C,),
}


def build_program(fat=BF16):
    from contextlib import ExitStack

    nc = bacc.Bacc("TRN2", target_bir_lowering=False, debug=False, num_devices=B)
    aps = {}
    for name, shape in INPUT_SPECS.items():
        aps[name] = nc.dram_tensor(name, list(shape), F32, kind="ExternalInput").ap()
    y = nc.dram_tensor("y", [C, P], F32, kind="ExternalOutput").ap()
    with tile.TileContext(nc) as tc:
        with ExitStack() as ctx:
            build_core_kernel(ctx, tc, y, aps, fat=fat)
    nc.compile()
    return nc


def kernel(**inputs):
    ins = {k: np.ascontiguousarray(np.asarray(v, dtype=np.float32)) for k, v in inputs.items()}
    nc = build_program()
    in_maps = []
    for b in range(B):
        m = {k: ins[k] for k in INPUT_SPECS if k != "u_history"}
        m["u_history"] = ins["u_history"][b]
        in_maps.append(m)
    res = run_bass_kernel_spmd(nc, in_maps, list(range(B)))
    out = np.stack([r["y"] for r in res.results], axis=0)
    return out.astype(np.float32)


if __name__ == "__main__":
    rng = np.random.default_rng(0)
    ins = {k: rng.standard_normal(v).astype(np.float32) * 0.05 for k, v in INPUT_SPECS.items()}
    out = kernel(**ins)
    print(out.shape, out.dtype)


# revision 66
# speedup vs baseline: 10.1374x; 10.1374x over previous
"""Trainium2 Bass kernel for CausalTemporalAttention.

Math (per batch b, position p; all ops pointwise in p except the k=3 conv):
  feat[w,c,p] = gelu(conv1d(u[w])[c,p] + emb_b[c])          (W=16, C=64, P=8192)
  last = feat[W-1]
  Q = q_w @ last + q_b
  scores[w,p] = sum_c K[w,c,p]*Q[c,p] / 8
              = (sum_c feat[w,c,p]*Qk[c,p] + sum_c k_b[c]*Q[c,p]) / 8
                where Qk = k_w^T @ Q            <- K projection eliminated
  attn = softmax_w(scores);   es = exp(s) = (1+tanh(s/2))/(1-tanh(s/2))
  ctxF[c,p] = sum_w attn[w,p]*feat[w,c,p]
  out = (o_w@v_w) @ ctxF + (o_w@v_b + o_b)     <- V projection eliminated
  x = out + last;  y = GroupNorm4(x)*gamma + beta

Sharding: data-parallel over B (8 batches -> 8 cores), no collectives.
"""

import numpy as np

import concourse.bass as bass
import concourse.tile as tile
from concourse import bacc, mybir
from concourse.bass_utils import run_bass_kernel_spmd
from concourse.masks import make_identity

F32 = mybir.dt.float32
BF16 = mybir.dt.bfloat16
AF = mybir.ActivationFunctionType
OP = mybir.AluOpType

B, W, P, C, G = 8, 16, 8192, 64, 4
EPS = 1e-5
PC = 2048          # positions per chunk
NS = 512           # matmul free-dim split (one PSUM bank fp32)
NCHUNK = P // PC
NPAIR = W // 2
NSPL = PC // NS

# pair -> (w_top, w_bot); pair 7 is (15, 14) so that `last` (w=15) sits in
# partitions 0:64 of its feat tile (matmul rhs must start at partition 0).
PAIR_W = [(2 * i, 2 * i + 1) for i in range(NPAIR - 1)] + [(15, 14)]


def build_core_kernel(ctx, tc, y_ap, ins, fat=BF16):
    """Emit the per-core program. ins: dict name -> DRAM AP. y_ap: (C, P) f32."""
    nc = tc.nc

    consts = ctx.enter_context(tc.tile_pool(name="consts", bufs=1))
    stage = ctx.enter_context(tc.tile_pool(name="stage", bufs=2))

    featp = ctx.enter_context(tc.tile_pool(name="featp", bufs=18))
    onep = ctx.enter_context(tc.tile_pool(name="onep", bufs=2))
    fatp = ctx.enter_context(tc.tile_pool(name="fatp", bufs=4))
    esbp = ctx.enter_context(tc.tile_pool(name="esbp", bufs=3))
    smallp = ctx.enter_context(tc.tile_pool(name="smallp", bufs=2))
    persist = ctx.enter_context(tc.tile_pool(name="persist", bufs=1))
    outp = ctx.enter_context(tc.tile_pool(name="outp", bufs=1))
    dramp = ctx.enter_context(tc.tile_pool(name="dramp", bufs=2, space="DRAM"))

    pp_conv = ctx.enter_context(tc.tile_pool(name="pp_conv", bufs=2, space="PSUM"))
    pp_mm = ctx.enter_context(tc.tile_pool(name="pp_mm", bufs=1, space="PSUM"))
    pp_tp = ctx.enter_context(tc.tile_pool(name="pp_tp", bufs=1, space="PSUM"))

    # ---------------- constants / weights prep ----------------
    # conv lhsT block (6, 128): row j*3+t, col w'*64+c = emb_w[c,t] if w'==j
    # else 0; replicated at partition strips 0/32/64/96 (rhs/lhsT partition
    # bases must be 32-aligned for compute engines).
    convW4 = consts.tile([128, 128], F32, tag="convW4")
    nc.vector.memset(convW4[:], 0.0)
    emb_tc = ins["emb_w"].rearrange("c one t -> (one t) c")  # (3, 64) strided
    for q in range(4):
        for j in range(2):
            nc.sync.dma_start(
                out=convW4[32 * q + 3 * j : 32 * q + 3 * j + 3, 64 * j : 64 * j + 64],
                in_=emb_tc,
            )

    # rounded (f32r) conv weights: fp32r matmul inputs must be produced by
    # a rounding instruction
    convW4_r = consts.tile([128, 128], mybir.dt.float32r, tag="convW4_r")
    nc.scalar.copy(out=convW4_r[:], in_=convW4[:])

    # identity (used by PE transposes); bf16 variant for bf16 transposes
    I128 = consts.tile([128, 128], F32, tag="I128")
    make_identity(nc, I128[:])
    Ib16 = consts.tile([128, 128], BF16, tag="Ib16")
    nc.vector.tensor_copy(out=Ib16[:], in_=I128[:])

    def load_small(name, shape, tag):
        t = consts.tile(shape, F32, tag=tag)
        src = ins[name]
        if len(shape) == 2 and shape[1] == 1 and len(src.shape) == 1:
            src = src.unsqueeze(1)
        nc.sync.dma_start(out=t[:], in_=src)
        return t

    qb1 = load_small("q_b", [C, 1], "qb1")
    kb1 = load_small("k_b", [C, 1], "kb1")
    vb1 = load_small("v_b", [C, 1], "vb1")
    ob1 = load_small("o_b", [C, 1], "ob1")
    gamma1 = load_small("gn_gamma", [C, 1], "gamma1")
    beta1 = load_small("gn_beta", [C, 1], "beta1")
    vw_sb = load_small("v_w", [C, C], "vw")  # v_w[m, c] as-is
    qw_sb = load_small("q_w", [C, C], "qw")
    kw_sb = load_small("k_w", [C, C], "kw")

    embB2 = consts.tile([128, 1], F32, tag="embB2")
    for j in range(2):
        nc.sync.dma_start(out=embB2[64 * j : 64 * j + 64, :], in_=ins["emb_b"].unsqueeze(1))

    owT_f = consts.tile([C, C], F32, tag="owT")
    nc.sync.dma_start(out=owT_f[:], in_=ins["o_w"].rearrange("o c -> c o"))

    # Fold q then k projections into one matrix: Qk[o,p] = sum_c' M[o,c']*last[c',p]
    # with M = k_w^T q_w; MT[c',o] = sum_c q_w[c,c']*k_w[c,o].
    mt_ps = pp_mm.tile([C, C], F32, tag="cx", bufs=NSPL, name="mt_ps")
    nc.tensor.matmul(out=mt_ps[:], lhsT=qw_sb[:], rhs=kw_sb[:], start=True, stop=True)
    # lhsT_M (64, 128): MT duplicated along free (both c-halves of qk2)
    lhsT_M = consts.tile([C, 128], fat, tag="lhsT_M")
    nc.vector.tensor_copy(out=lhsT_M[:, 0:C], in_=mt_ps[:])
    nc.vector.tensor_copy(out=lhsT_M[:, C : 2 * C], in_=mt_ps[:])
    # kqb = k_w^T q_b, duplicated over both halves -> per-partition bias of qk2
    kqb_ps = pp_mm.tile([C, 1], F32, tag="cx", bufs=NSPL, name="kqb_ps")
    nc.tensor.matmul(out=kqb_ps[:], lhsT=kw_sb[:], rhs=qb1[:], start=True, stop=True)
    kqb2 = consts.tile([128, 1], F32, tag="kqb2")
    nc.vector.tensor_copy(out=kqb2[0:C, :], in_=kqb_ps[:])
    nc.sync.dma_start(out=kqb2[C : 2 * C, :], in_=kqb2[0:C, :])
    # qTkb = q_w^T k_b -> scores kb-term uses `last` directly;
    # kb16 (64, 16) = qTkb broadcast along free
    qtkb_ps = pp_mm.tile([C, 1], F32, tag="cx", bufs=NSPL, name="qtkb_ps")
    nc.tensor.matmul(out=qtkb_ps[:], lhsT=qw_sb[:], rhs=kb1[:], start=True, stop=True)
    qtkb = consts.tile([C, 1], F32, tag="qtkb")
    nc.vector.tensor_copy(out=qtkb[:], in_=qtkb_ps[:])
    kb16 = consts.tile([C, W], fat, tag="kb16")
    nc.vector.memset(kb16[:], 0.0)
    nc.vector.tensor_scalar(
        out=kb16[:], in0=kb16[:], scalar1=qtkb[:], scalar2=None, op0=OP.add
    )
    # kbqb = (k_b . q_b)/16: constant bias added to all scores inside tanh
    kbqb_ps = pp_mm.tile([1, 1], F32, tag="cx", bufs=NSPL, name="kbqb_ps")
    nc.tensor.matmul(out=kbqb_ps[:], lhsT=kb1[:], rhs=qb1[:], start=True, stop=True)
    kbqb1 = consts.tile([1, 1], F32, tag="kbqb1")
    nc.scalar.mul(out=kbqb1[:], in_=kbqb_ps[:], mul=1.0 / 16.0)
    kbqb128 = consts.tile([128, 1], F32, tag="kbqb128")
    nc.gpsimd.partition_broadcast(out_ap=kbqb128[:], in_ap=kbqb1[:])

    # per-pair scores selector (128, 16)
    ones16 = []
    for i in range(NPAIR):
        wt, wb = PAIR_W[i]
        t = consts.tile([128, W], fat, tag=f"ones16_{i}")
        nc.vector.memset(t[:], 0.0)
        nc.vector.memset(t[0:64, wt : wt + 1], 1.0)
        nc.vector.memset(t[64:128, wb : wb + 1], 1.0)
        ones16.append(t)

    # pair-sum reduction matrix [I64; I64] (128, 64)
    sumI = consts.tile([128, C], fat, tag="sumI")
    nc.vector.tensor_copy(out=sumI[0:64, :], in_=I128[0:64, 0:64])
    nc.vector.tensor_copy(out=sumI[64:128, :], in_=I128[64:128, 64:128])

    # OV^T = (o_w @ v_w)^T : OVT[c,o] = sum_m v_w[m,c] * o_w[o,m]
    ovT_ps = pp_mm.tile([C, C], F32, tag="cx", bufs=NSPL)
    nc.tensor.matmul(out=ovT_ps[:], lhsT=vw_sb[:], rhs=owT_f[:], start=True, stop=True)
    ovT = consts.tile([C, C], fat, tag="ovT")
    nc.vector.tensor_copy(out=ovT[:], in_=ovT_ps[:])

    # x shift vector: o_w @ v_b + o_b
    ovb_ps = pp_mm.tile([C, 1], F32, tag="cx", bufs=NSPL)
    nc.tensor.matmul(out=ovb_ps[:], lhsT=owT_f[:], rhs=vb1[:], start=True, stop=True)
    sxv = consts.tile([C, 1], F32, tag="sxv")
    nc.vector.tensor_add(out=sxv[:], in0=ovb_ps[:], in1=ob1[:])

    # group-combine matrices for GroupNorm
    CG = C // G
    # gsel[c, g] = 1/CG where c//CG == g (via two affine selects: 0<=c-CG*g<CG)
    gsel = consts.tile([C, G], F32, tag="gsel")
    nc.gpsimd.memset(gsel[:], 1.0 / CG)
    nc.gpsimd.affine_select(
        out=gsel[:], in_=gsel[:], compare_op=OP.is_ge, fill=0.0,
        base=0, channel_multiplier=1, pattern=[[-CG, G]],
    )
    nc.gpsimd.affine_select(
        out=gsel[:], in_=gsel[:], compare_op=OP.is_ge, fill=0.0,
        base=CG - 1, channel_multiplier=-1, pattern=[[CG, G]],
    )
    # gselT[g, c] = 1.0 where c//CG == g
    gselT = consts.tile([G, C], F32, tag="gselT")
    nc.gpsimd.memset(gselT[:], 1.0)
    nc.gpsimd.affine_select(
        out=gselT[:], in_=gselT[:], compare_op=OP.is_ge, fill=0.0,
        base=0, channel_multiplier=-CG, pattern=[[1, C]],
    )
    nc.gpsimd.affine_select(
        out=gselT[:], in_=gselT[:], compare_op=OP.is_ge, fill=0.0,
        base=CG - 1, channel_multiplier=CG, pattern=[[-1, C]],
    )

    # persistent accumulators
    x_all = persist.tile([C, P], fat, tag="x_all")
    stats_all = persist.tile([C, NCHUNK * NSPL, 6], F32, tag="stats_all")
    # u staging tile (persistent; strip rows 6..31 stay zero so the rounded
    # 32-row matmul strip reads defined zeros that the zero lhsT rows kill)
    ut = persist.tile([128, 2 * PC], F32, tag="ut")
    nc.vector.memset(ut[:], 0.0)
    ut_r = persist.tile([128, 2 * PC], mybir.dt.float32r, tag="ut_r")

    # u padded with one zero column on each side (conv pad=1), staged in DRAM
    u_pad = dramp.tile([W, P + 2], F32, tag="u_pad")
    zcol = consts.tile([W, 1], F32, tag="zcol")
    nc.vector.memset(zcol[:], 0.0)
    nc.sync.dma_start(out=u_pad[:, 0:1], in_=zcol[:])
    nc.sync.dma_start(out=u_pad[:, P + 1 : P + 2], in_=zcol[:])
    nc.sync.dma_start(out=u_pad[:, 1 : P + 1], in_=ins["u_history"])
    u = u_pad[:]  # (W, P+2); u_pad[:, p+1] == u[:, p]

    # ---------------- main loop over P chunks (2-stage sw pipeline) ----------
    # Stage A(ck): u-load, conv, gelu, Q/Qk2.  Stage B(ck): scores, softmax,
    # attention-weighted sum, output, bn stats.  Emission order A0 A1 B0 A2 B1
    # A3 B2 B3 lets chunk ck+1's conv/gelu fill engine idle during chunk ck's
    # DMA-broadcast / softmax phases.
    dma_engines = [nc.sync, nc.gpsimd, nc.scalar]

    def stage_a(ck):
        p0 = ck * PC
        # one u tile: pair i at partition strip 32*(i%4), cols (i//4)*PC,
        # rows 32q + 3j + t = u_pad[w(i,j), p0+p+t] (pad shift: u[p] = u_pad[p+1])
        # One DMA per (pair, j): dest = 3 contiguous partitions (taps), src AP
        # partition dim = tap shift (step 1 in u_pad's free axis).
        for i in range(NPAIR):
            a, q = divmod(i, 4)
            for j in range(2):
                wij = PAIR_W[i][j]
                src = bass.AP(
                    tensor=u.tensor,
                    offset=u.offset + wij * (P + 2) + p0,
                    ap=[[1, 3], [1, PC]],
                )
                dma_engines[(2 * i + j) % len(dma_engines)].dma_start(
                    out=ut[32 * q + 3 * j : 32 * q + 3 * j + 3, a * PC : (a + 1) * PC],
                    in_=src,
                )
        nc.scalar.copy(out=ut_r[:], in_=ut[:])

        # conv + gelu; emit pair 7 first so Q/Qk can start early
        feat = [None] * NPAIR
        # strip-grouped so consecutive pairs share the conv lhsT (LDW elision)
        pair_order = [7, 3, 0, 4, 1, 5, 2, 6]
        for i in pair_order:
            a, q = divmod(i, 4)
            ft = featp.tile([128, PC], fat, tag="feat", name=f"ft{ck}_{i}")
            for k in range(NSPL):
                cps = pp_conv.tile([128, NS], F32, tag="conv", name=f"cps{ck}_{i}_{k}")
                # float32r: full-rate fp32 matmul (TF32-ish multiply), same bytes
                nc.tensor.matmul(
                    out=cps[:],
                    lhsT=convW4_r[32 * q : 32 * q + 6, :],
                    rhs=ut_r[
                        32 * q : 32 * q + 6, a * PC + k * NS : a * PC + (k + 1) * NS
                    ],
                    start=True,
                    stop=True,
                    tile_position=(32 * q, 0),
                )
                nc.scalar.activation(
                    out=ft[:, k * NS : (k + 1) * NS],
                    in_=cps[:],
                    func=AF.Gelu,
                    bias=embB2[:],
                    scale=1.0,
                )
            feat[i] = ft

        # Qk2 = [M @ last + kqb; same] in one matmul per split (M = k_w^T q_w)
        qk2 = fatp.tile([128, PC], fat, tag="qk2", bufs=3, name=f"qk2_{ck}")
        for k in range(NSPL):
            qps = pp_mm.tile([128, NS], F32, tag="mm", bufs=1, name=f"qps{ck}_{k}")
            nc.tensor.matmul(
                out=qps[:],
                lhsT=lhsT_M[:],
                rhs=feat[7][0:64, k * NS : (k + 1) * NS],
                start=True,
                stop=True,
            )
            nc.vector.tensor_scalar(
                out=qk2[:, k * NS : (k + 1) * NS],
                in0=qps[:],
                scalar1=kqb2[:],
                scalar2=None,
                op0=OP.add,
            )
        return feat, qk2

    def stage_b1(ck, feat, qk2):
        p0 = ck * PC
        # prod = feat[i] * qk2 (per split) ; scores via accumulating matmuls
        s_sb = onep.tile([W, PC], fat, tag="s_sb", name=f"s_sb{ck}")
        prods = {}
        for k in range(NSPL):
            sl = slice(k * NS, (k + 1) * NS)
            scps = pp_mm.tile([W, NS], F32, tag="cx", bufs=NSPL, name=f"scps{ck}_{k}")
            for i in range(NPAIR):
                if k == 0:
                    pr = fatp.tile([128, PC], fat, tag="prod", bufs=2, name=f"pr{ck}_{i}")
                    nc.vector.tensor_mul(out=pr[:], in0=feat[i][:], in1=qk2[:])
                    prods[i] = pr
                nc.tensor.matmul(
                    out=scps[:],
                    lhsT=ones16[i][:],
                    rhs=prods[i][:, sl],
                    start=(i == 0),
                    stop=False,
                )
            nc.tensor.matmul(
                out=scps[:],
                lhsT=kb16[:],
                rhs=feat[7][0:64, sl],
                start=False,
                stop=True,
            )
            nc.any.tensor_copy(out=s_sb[:, sl], in_=scps[:])

        # transpose scores to (128, 16 slabs, 16 w) and do softmax there
        sT_ps = pp_tp.tile([128, W * W], fat, tag="tp", name=f"sT_ps{ck}")
        for s in range(W):
            nc.tensor.transpose(
                out=sT_ps[:, W * s : W * (s + 1)],
                in_=s_sb[:, 128 * s : 128 * (s + 1)],
                identity=Ib16[0:16, 0:16],
            )
        # es = exp(s/8) = (1+th)/(1-th), th = tanh(s/16)
        th = smallp.tile([128, W * W], F32, tag="th", name=f"th{ck}")
        nc.scalar.activation(
            out=th[:], in_=sT_ps[:], func=AF.Tanh, scale=1.0 / 16.0, bias=kbqb128[:]
        )
        num = smallp.tile([128, W * W], F32, tag="num", name=f"num{ck}")
        nc.scalar.activation(out=num[:], in_=th[:], func=AF.Identity, bias=1.0)
        den = smallp.tile([128, W * W], F32, tag="den", name=f"den{ck}")
        nc.scalar.activation(out=den[:], in_=th[:], func=AF.Identity, bias=1.0, scale=-1.0)
        nc.vector.reciprocal(out=den[:], in_=den[:])
        es = num  # es = (1+th)*rden, in place
        nc.vector.tensor_mul(out=es[:], in0=num[:], in1=den[:])
        es3 = es[:].rearrange("p (t w) -> p t w", w=W)
        D = smallp.tile([128, W], F32, tag="D", name=f"D{ck}")
        nc.vector.tensor_reduce(out=D[:], in_=es3, axis=mybir.AxisListType.X, op=OP.add)
        nc.vector.reciprocal(out=D[:], in_=D[:])
        attnT = smallp.tile([128, W * W], fat, tag="attnT", name=f"attnT{ck}")
        nc.vector.tensor_mul(
            out=attnT[:].rearrange("p (t w) -> p t w", w=W),
            in0=es3,
            in1=D[:].unsqueeze(2).to_broadcast([128, W, W]),
        )

        # transpose attn back to (16, PC), stage to DRAM for broadcast
        attn_sb = smallp.tile([W, PC], fat, tag="attn_sb", name=f"attn_sb{ck}")
        for k in range(NSPL):
            atps = pp_tp.tile([W, NS], fat, tag="tp", name=f"atps{ck}_{k}")
            for s in range(4):
                g = 4 * k + s
                nc.tensor.transpose(
                    out=atps[:, 128 * s : 128 * (s + 1)],
                    in_=attnT[:, W * g : W * (g + 1)],
                    identity=Ib16[:],
                )
            nc.any.tensor_copy(out=attn_sb[:, k * NS : (k + 1) * NS], in_=atps[:])
        attn_dram = dramp.tile([W, PC], fat, tag="attn_dram", name=f"attn_dram{ck}")
        nc.sync.dma_start(out=attn_dram[:], in_=attn_sb[:])
        return attn_dram

    def stage_b2(ck, feat, qk2, attn_dram):
        p0 = ck * PC
        # ctxF = sum_w attn_w * feat_w  (broadcast attn rows via DMA, multiply,
        # then pair-sum with accumulating [I64;I64] matmuls). Pair-outer order
        # releases feat[i]/esB early; the 4 split accumulators stay live.
        ctxF = smallp.tile([C, PC], fat, tag="ctxF", name=f"ctxF{ck}")
        cxps = [
            pp_mm.tile([C, NS], F32, tag="cx", bufs=NSPL, name=f"cx{ck}_{_k}")
            for _k in range(NSPL)
        ]
        for i in range(NPAIR):
            wt, wb = PAIR_W[i]
            esB = esbp.tile([128, PC], fat, tag="esB", name=f"esB{ck}_{i}")
            dma_engines[(2 * i) % len(dma_engines)].dma_start(
                out=esB[0:64, :],
                in_=attn_dram[wt : wt + 1, :].to_broadcast([64, PC]),
            )
            dma_engines[(2 * i + 1) % len(dma_engines)].dma_start(
                out=esB[64:128, :],
                in_=attn_dram[wb : wb + 1, :].to_broadcast([64, PC]),
            )
            p2 = fatp.tile([128, PC], fat, tag="prod2", bufs=2, name=f"p2_{ck}_{i}")
            nc.vector.tensor_mul(out=p2[:], in0=feat[i][:], in1=esB[:])
            for k in range(NSPL):
                sl = slice(k * NS, (k + 1) * NS)
                nc.tensor.matmul(
                    out=cxps[k][:],
                    lhsT=sumI[:],
                    rhs=p2[:, sl],
                    start=(i == 0),
                    stop=(i == NPAIR - 1),
                )
        for k in range(NSPL):
            sl = slice(k * NS, (k + 1) * NS)
            nc.any.tensor_copy(out=ctxF[:, sl], in_=cxps[k][:])

        # out = OV @ ctxF ; x = out + sxv + last ; bn stats
        for k in range(NSPL):
            ops_ = pp_mm.tile([C, NS], F32, tag="mm", bufs=1, name=f"ops{ck}_{k}")
            nc.tensor.matmul(
                out=ops_[:],
                lhsT=ovT[:],
                rhs=ctxF[:, k * NS : (k + 1) * NS],
                start=True,
                stop=True,
            )
            xsl = x_all[:, p0 + k * NS : p0 + (k + 1) * NS]
            nc.vector.scalar_tensor_tensor(
                out=xsl,
                in0=ops_[:],
                scalar=sxv[:],
                in1=feat[7][0:64, k * NS : (k + 1) * NS],
                op0=OP.add,
                op1=OP.add,
            )
            nc.vector.bn_stats(out=stats_all[:, ck * NSPL + k, :], in_=xsl)

    # 3-deep software pipeline: A(c+2) || B1(c+1) || B2(c)
    st = {}
    ad = {}
    st[0] = stage_a(0)
    st[1] = stage_a(1)
    ad[0] = stage_b1(0, *st[0])
    for ck in range(2, NCHUNK):
        st[ck] = stage_a(ck)
        ad[ck - 1] = stage_b1(ck - 1, *st[ck - 1])
        stage_b2(ck - 2, *st.pop(ck - 2), ad.pop(ck - 2))
    ad[NCHUNK - 1] = stage_b1(NCHUNK - 1, *st[NCHUNK - 1])
    stage_b2(NCHUNK - 2, *st.pop(NCHUNK - 2), ad.pop(NCHUNK - 2))
    stage_b2(NCHUNK - 1, *st.pop(NCHUNK - 1), ad.pop(NCHUNK - 1))

    # ---------------- GroupNorm finale ----------------
    mv = smallp.tile([C, 2], F32, tag="mv")
    nc.vector.bn_aggr(out=mv[:], in_=stats_all[:])
    # me2 = [mean, E[x^2]] per channel; group stats = 1/CG average via gsel
    me2 = smallp.tile([C, 2], F32, tag="me2")
    nc.any.tensor_copy(out=me2[:, 0:1], in_=mv[:, 0:1])
    sq = smallp.tile([C, 1], F32, tag="sq")
    nc.vector.tensor_mul(out=sq[:], in0=mv[:, 0:1], in1=mv[:, 0:1])
    nc.vector.tensor_add(out=me2[:, 1:2], in0=mv[:, 1:2], in1=sq[:])
    gm_ps = pp_mm.tile([G, 2], F32, tag="cx", bufs=NSPL)
    nc.tensor.matmul(out=gm_ps[:], lhsT=gsel[:], rhs=me2[:], start=True, stop=True)
    gm = smallp.tile([G, 2], F32, tag="gm")
    nc.any.tensor_copy(out=gm[:], in_=gm_ps[:])
    # rstd = 1/sqrt(var+eps); var = Ex2G - meanG^2  (use sqrt(eps - (m^2-Ex2)))
    nvar = smallp.tile([G, 1], F32, tag="nvar")
    nc.vector.scalar_tensor_tensor(
        out=nvar[:],
        in0=gm[:, 0:1],
        scalar=gm[:, 0:1],
        in1=gm[:, 1:2],
        op0=OP.mult,
        op1=OP.subtract,
    )
    eps_t = smallp.tile([G, 1], F32, tag="eps_t")
    nc.vector.memset(eps_t[:], EPS)
    std = smallp.tile([G, 1], F32, tag="std")
    nc.scalar.activation(out=std[:], in_=nvar[:], func=AF.Sqrt, scale=-1.0, bias=eps_t[:])
    rstd = smallp.tile([G, 1], F32, tag="rstd")
    nc.vector.reciprocal(out=rstd[:], in_=std[:])
    # broadcast group [mean, rstd] to channels
    grs = smallp.tile([G, 2], F32, tag="grs")
    nc.any.tensor_copy(out=grs[:, 0:1], in_=gm[:, 0:1])
    nc.any.tensor_copy(out=grs[:, 1:2], in_=rstd[:])
    chb_ps = pp_mm.tile([C, 2], F32, tag="cx", bufs=NSPL)
    nc.tensor.matmul(out=chb_ps[:], lhsT=gselT[:], rhs=grs[:], start=True, stop=True)
    chb = smallp.tile([C, 2], F32, tag="chb")
    nc.any.tensor_copy(out=chb[:], in_=chb_ps[:])
    scale_v = smallp.tile([C, 1], F32, tag="scale_v")
    nc.vector.tensor_mul(out=scale_v[:], in0=chb[:, 1:2], in1=gamma1[:])
    nmean = smallp.tile([C, 1], F32, tag="nmean")
    nc.vector.tensor_mul(out=nmean[:], in0=chb[:, 0:1], in1=scale_v[:])
    bias_v = smallp.tile([C, 1], F32, tag="bias_v")
    nc.vector.tensor_sub(out=bias_v[:], in0=beta1[:], in1=nmean[:])

    HPC = PC // 2
    for h in range(2 * NCHUNK):
        off = h * HPC
        ysl = outp.tile([C, HPC], F32, tag="y_sb", bufs=2, name=f"ysl{h}")
        nc.vector.tensor_scalar(
            out=ysl[:],
            in0=x_all[:, off : off + HPC],
            scalar1=scale_v[:],
            scalar2=bias_v[:],
            op0=OP.mult,
            op1=OP.add,
        )
        dma_engines[h % len(dma_engines)].dma_start(
            out=y_ap[:, off : off + HPC], in_=ysl[:]
        )


INPUT_SPECS = {
    "u_history": (W, P),
    "emb_w": (C, 1, 3),
    "emb_b": (C,),
    "q_w": (C, C),
    "q_b": (C,),
    "k_w": (C, C),
    "k_b": (C,),
    "v_w": (C, C),
    "v_b": (C,),
    "o_w": (C, C),
    "o_b": (C,),
    "gn_gamma": (C,),
    "gn_beta": (C,),
}


def build_program(fat=BF16):
    from contextlib import ExitStack

    nc = bacc.Bacc("TRN2", target_bir_lowering=False, debug=False, num_devices=B)
    aps = {}
    for name, shape in INPUT_SPECS.items():
        aps[name] = nc.dram_tensor(name, list(shape), F32, kind="ExternalInput").ap()
    y = nc.dram_tensor("y", [C, P], F32, kind="ExternalOutput").ap()
    with tile.TileContext(nc) as tc:
        with ExitStack() as ctx:
            build_core_kernel(ctx, tc, y, aps, fat=fat)
    nc.compile()
    return nc


def kernel(**inputs):
    ins = {k: np.ascontiguousarray(np.asarray(v, dtype=np.float32)) for k, v in inputs.items()}
    nc = build_program()
    in_maps = []
    for b in range(B):
        m = {k: ins[k] for k in INPUT_SPECS if k != "u_history"}
        m["u_history"] = ins["u_history"][b]
        in_maps.append(m)
    res = run_bass_kernel_spmd(nc, in_maps, list(range(B)))
    out = np.stack([r["y"] for r in res.results], axis=0)
    return out.astype(np.float32)


if __name__ == "__main__":
    rng = np.random.default_rng(0)
    ins = {k: rng.standard_normal(v).astype(np.float32) * 0.05 for k, v in INPUT_SPECS.items()}
    out = kernel(**ins)
    print(out.shape, out.dtype)
